# revision 25
# baseline (speedup 1.0000x reference)
"""Trainium2 Bass kernel for AdvancedGATModel (4-layer edge-featured GAT +
Set2Set pooling + MLP head), sharded across 8 NeuronCores.

Sharding: nodes are split into 8 contiguous slices (6250 each); each core owns
the edges whose *destination* lands in its slice (plus self-loops), so segment
softmax and the scatter-add aggregation are core-local.  Per layer each core
computes the linear transform of its node slice, all cores AllGather the
transformed features (bf16) into a replicated table, and each core gathers its
edges' source rows via indirect DMA.  Segment softmax/weighted-sum run as
dense 128-edge-tile matmuls against host-precomputed fp8 indicator matrices
(edges x window-nodes and its transpose).  Per-edge "a_edge" attention terms
are host-precomputed per layer.  Features use a head-minor (c-major) column
layout so the per-edge softmax scaling runs in the DVE 2x packed mode.  The
linear transform of layer l+1 is fused into layer l's per-window epilogue, so
no transposed-feature round trip through DRAM is needed.  Set2Set is sharded
by graph (64 graphs/core; graphs never straddle cores) and runs in
bf16/f32r, and the [64,5] head outputs are concatenated on host.

The program is identical on all 8 cores (SPMD); only input *data* differs.
All shapes below are hardcoded for the grading problem.
"""

import numpy as np

import concourse.bass as bass
import concourse.bacc as bacc
import concourse.tile as tile
import concourse.mybir as mybir
from concourse.bass_utils import run_bass_kernel_spmd

F32 = mybir.dt.float32
F32R = mybir.dt.float32r
BF = mybir.dt.bfloat16
F8 = mybir.dt.float8e4
I32 = mybir.dt.int32
AF = mybir.ActivationFunctionType
OP = mybir.AluOpType
P = 128

NPBF = mybir.dt.np(BF)
NPF8 = mybir.dt.np(F8)

HS = [8, 8, 8, 1]          # heads per layer
# stored feature j = c*8+h  <->  reference feature h*32+c  (layers 0-2)
PSTORE = np.array([h * 32 + c for c in range(32) for h in range(8)], np.int64)


class CFG:
    # full problem; small-mode tests override these
    N = 50000          # nodes
    E = 800000         # edges (before self loops)
    G = 512            # graphs
    ND = 14            # node feat dim
    ED = 4             # edge feat dim
    GD = 13            # global feat dim
    D = 256            # hidden
    H = 8              # heads
    C = 32             # per-head channels
    NC = 8             # cores
    NPC = N // NC      # nodes per core = 6250
    W = 49             # node windows per core (ceil(NPC/128))
    GPC = G // NC      # graphs per core = 64
    TS = 51            # set2set node tiles per core (capacity TS*128 nodes)
    S2S_STEPS = 3
    LAYERS = 4
    FAKE_AG = False    # replace AllGathers with local copies (debug only)
    # timing-attribution probes (wrong outputs; timing only)
    SKIP_GATHER = False
    SKIP_MMT = False
    SKIP_WFEX = False
    SKIP_NU = False
    # set by host_prep:
    KS = None          # per-window edge tile counts (common across cores)
    SK = None          # sum(KS)
    KM = None          # max(KS)
    OFFS = None        # cumulative offsets

    @classmethod
    def derive(cls):
        cls.NPC = cls.N // cls.NC
        cls.GPC = cls.G // cls.NC
        cls.W = -(-cls.NPC // P)
        return cls


def make_small_cfg():
    class Small(CFG):
        N = 2048
        E = 8192
        G = 64
        W = 2
        TS = 3
    return Small.derive()


# ------------------------------------------------------------------
# host-side preprocessing
# ------------------------------------------------------------------

def host_prep(inp, cfg):
    """Build per-core input maps from the full input dict."""
    N, E, G = cfg.N, cfg.E, cfg.G
    NC, NPC, W, GPC, TS = cfg.NC, cfg.NPC, cfg.W, cfg.GPC, cfg.TS
    D, ED, GD = cfg.D, cfg.ED, cfg.GD

    src = np.asarray(inp["edge_index"][0])
    dst = np.asarray(inp["edge_index"][1])
    ea = np.asarray(inp["edge_attr"], dtype=np.float32)
    batch = np.asarray(inp["batch_idx"])
    x = np.asarray(inp["x"], dtype=np.float32)

    # self-loop attr = mean incoming edge attr (0 for isolated nodes)
    deg = np.bincount(dst, minlength=N).astype(np.float32)
    loop = np.zeros((N, ED), np.float32)
    for j in range(ED):
        loop[:, j] = np.bincount(dst, weights=ea[:, j], minlength=N)
    loop /= np.maximum(deg, 1.0)[:, None]

    src2 = np.concatenate([src, np.arange(N, dtype=np.int64)])
    dst2 = np.concatenate([dst, np.arange(N, dtype=np.int64)])
    ea2 = np.concatenate([ea, loop], axis=0).astype(np.float32)

    order = np.argsort(dst2, kind="stable")
    s_src = src2[order]
    s_dst = dst2[order]
    s_ea = ea2[order]

    # per-layer weights: head-minor (c-major) permutation of 256-wide dims
    Wms, asrcs, adsts, biases, aEs = [], [], [], [], []
    for i in range(4):
        Wm = np.asarray(inp[f"g{i}_W"], np.float32)          # [din, H*C]
        We = np.asarray(inp[f"g{i}_We"], np.float32)         # [ED, H*C]
        asrc = np.asarray(inp[f"g{i}_asrc"], np.float32)     # [h, c]
        adst = np.asarray(inp[f"g{i}_adst"], np.float32)
        aedge = np.asarray(inp[f"g{i}_aedge"], np.float32)
        b = np.asarray(inp[f"g{i}_b"], np.float32)
        h, c = asrc.shape
        if i > 0:
            Wm = Wm[PSTORE, :]          # input comes in stored (c-major) space
        af, df, bf = asrc.reshape(-1), adst.reshape(-1), b
        if i < 3:
            Wm = Wm[:, PSTORE]          # output in stored space
            af, df, bf = af[PSTORE], df[PSTORE], bf[PSTORE]
        Wms.append(Wm)
        asrcs.append(np.broadcast_to(af, (P, D)).copy())
        adsts.append(np.broadcast_to(df, (P, D)).copy())
        biases.append(np.broadcast_to(bf, (P, D)).copy())
        # per-edge attention term: aE[e, h] = sum_d ea2[e,d] * M_ae[d,h]
        M_ae = (We.reshape(ED, h, c) * aedge[None]).sum(-1)  # [ED, h]
        aEs.append((s_ea @ M_ae).astype(np.float32))         # [E2 sorted, h]

    # -------- per-core edge ranges and common per-window tile counts -----
    e_bounds = [(np.searchsorted(s_dst, c2 * NPC), np.searchsorted(s_dst, (c2 + 1) * NPC))
                for c2 in range(NC)]
    cnts = np.zeros((NC, W), np.int64)
    for ci in range(NC):
        e0, e1 = e_bounds[ci]
        cd = s_dst[e0:e1] - ci * NPC
        for w in range(W):
            lo, hi = w * P, min((w + 1) * P, NPC)
            cnts[ci, w] = np.searchsorted(cd, hi) - np.searchsorted(cd, lo)
    KS = [int(-(-cnts[:, w].max() // P)) for w in range(W)]
    KS = [max(k, 1) for k in KS]
    OFFS = np.concatenate([[0], np.cumsum(KS)]).astype(np.int64)
    SK = int(OFFS[-1])
    cfg.KS, cfg.SK, cfg.KM, cfg.OFFS = KS, SK, max(KS), OFFS

    # graph ranges per core for set2set (graphs never straddle cores)
    gbound = np.searchsorted(batch, np.arange(G + 1))

    in_maps = []
    for ci in range(NC):
        n0 = ci * NPC
        e0, e1 = e_bounds[ci]
        cs, cd = s_src[e0:e1], s_dst[e0:e1] - n0
        caE = [aE[e0:e1] for aE in aEs]

        idx_arr = np.zeros((P, SK), np.int32)
        dcol_arr = np.full((P, SK), 255.0, NPBF)
        drow_arr = np.full((1, SK * P), 255.0, NPBF)
        aE_arr = [np.zeros((P, SK, HS[l]), NPBF) for l in range(4)]
        for w in range(W):
            lo, hi = w * P, min((w + 1) * P, NPC)
            a = np.searchsorted(cd, lo)
            b2 = np.searchsorted(cd, hi)
            cnt = b2 - a
            assert cnt <= KS[w] * P
            js = np.arange(cnt)
            tk = js // P + OFFS[w]
            pp = js % P
            drel = (cd[a:b2] - lo).astype(np.int64)
            idx_arr[pp, tk] = cs[a:b2]
            dcol_arr[pp, tk] = drel.astype(NPBF)
            drow_arr[0, tk * P + pp] = drel.astype(NPBF)
            for l in range(4):
                aE_arr[l][pp, tk, :] = caE[l][a:b2].astype(NPBF)

        # set2set: node range + padding for this core's graphs
        g0 = ci * GPC
        gn0, gn1 = gbound[g0], gbound[g0 + GPC]
        ncnt = gn1 - gn0
        assert ncnt <= TS * P, f"s2s overflow core {ci}: {ncnt} > {TS*P}"
        s2s_idx = np.zeros((TS * P,), np.int32)
        s2s_idx[:ncnt] = np.arange(gn0, gn1, dtype=np.int32)
        s2s_brel = np.full((TS * P,), -1, np.int64)
        s2s_brel[:ncnt] = batch[gn0:gn1] - g0
        Mb = np.zeros((TS * P, GPC), NPF8)
        ii = np.arange(ncnt)
        Mb[ii, s2s_brel[:ncnt]] = 1.0
        Mb = Mb.reshape(TS, P, GPC)

        m = dict(
            xT=np.ascontiguousarray(x[n0:n0 + NPC].T).astype(NPBF),
            idx_in=idx_arr,
            dcol_in=dcol_arr, drow_in=drow_arr,
            iota_in=np.broadcast_to(
                np.arange(P, dtype=np.float32).astype(NPBF), (P, P)).copy(),
            iotac_in=np.arange(P, dtype=np.float32).astype(NPBF).reshape(P, 1),
            ident_in=np.eye(P, dtype=NPBF),
            ones_in=np.ones((1, P), NPBF),
            s2s_idx_in=np.ascontiguousarray(s2s_idx.reshape(TS, P).T),
            Mb_in=np.ascontiguousarray(Mb.transpose(1, 0, 2).reshape(P, TS * GPC)),
            MbT_in=np.ascontiguousarray(Mb.reshape(TS * P, GPC).T),
            gfT_in=np.ascontiguousarray(
                np.asarray(inp["global_features"], np.float32)[g0:g0 + GPC].T
            ).astype(NPBF),
        )
        for l in range(4):
            m[f"W{l}"] = Wms[l].astype(NPBF)
            m[f"asrcf{l}"] = asrcs[l].astype(NPBF)
            m[f"adstf{l}"] = adsts[l].astype(NPBF)
            m[f"bias{l}"] = biases[l].astype(NPBF)
            m[f"aE{l}"] = aE_arr[l]
        m["WihT"] = np.ascontiguousarray(np.asarray(inp["s2s_Wih"], np.float32).T).astype(NPBF)
        m["WhhT"] = np.ascontiguousarray(np.asarray(inp["s2s_Whh"], np.float32).T).astype(NPBF)
        m["s2s_bias"] = (np.asarray(inp["s2s_bih"], np.float32)
                         + np.asarray(inp["s2s_bhh"], np.float32)).reshape(1, -1).astype(NPBF)
        m["p1W"] = np.asarray(inp["p1_W"], np.float32).astype(NPBF)
        m["p1b"] = np.asarray(inp["p1_b"], np.float32).reshape(1, -1).astype(NPBF)
        m["p2W"] = np.asarray(inp["p2_W"], np.float32).astype(NPBF)
        m["p2b"] = np.asarray(inp["p2_b"], np.float32).reshape(1, -1).astype(NPBF)
        m["p3W"] = np.asarray(inp["p3_W"], np.float32).astype(NPBF)
        m["p3b"] = np.asarray(inp["p3_b"], np.float32).reshape(1, -1).astype(NPBF)
        in_maps.append(m)
    return in_maps


# ------------------------------------------------------------------
# device kernel builder
# ------------------------------------------------------------------

def build_kernel(cfg):
    N, NPC, W, GPC, TS, SK = cfg.N, cfg.NPC, cfg.W, cfg.GPC, cfg.TS, cfg.SK
    D, ND, GD = cfg.D, cfg.ND, cfg.GD

    nc = bacc.Bacc("TRN2", target_bir_lowering=False, debug=False,
                   num_devices=cfg.NC)

    TT = {}

    def din(name, shape, dt, kind="ExternalInput"):
        TT[name] = nc.dram_tensor(name, shape, dt, kind=kind)

    din("xT", [ND, NPC], BF)
    din("idx_in", [P, SK], I32)
    din("dcol_in", [P, SK], BF)
    din("drow_in", [1, SK * P], BF)
    din("iota_in", [P, P], BF)
    din("iotac_in", [P, 1], BF)
    din("ident_in", [P, P], BF)
    din("ones_in", [1, P], BF)
    for l in range(4):
        d0 = ND if l == 0 else D
        din(f"W{l}", [d0, D], BF)
        din(f"asrcf{l}", [P, D], BF)
        din(f"adstf{l}", [P, D], BF)
        din(f"bias{l}", [P, D], BF)
        din(f"aE{l}", [P, SK, HS[l]], BF)
    din("s2s_idx_in", [P, TS], I32)
    din("Mb_in", [P, TS * GPC], F8)
    din("MbT_in", [GPC, TS * P], F8)
    din("gfT_in", [GD, GPC], BF)
    din("WihT", [2 * D, 4 * D], BF)
    din("WhhT", [D, 4 * D], BF)
    din("s2s_bias", [1, 4 * D], BF)
    din("p1W", [2 * D + GD, D], BF)
    din("p1b", [1, D], BF)
    din("p2W", [D, D // 2], BF)
    din("p2b", [1, D // 2], BF)
    din("p3W", [D // 2, 5], BF)
    din("p3b", [1, 5], BF)
    din("out", [GPC, 5], F32, kind="ExternalOutput")

    with tile.TileContext(nc) as tc:
        build_body(nc, tc, cfg, TT)
    nc.compile()
    return nc


def build_body(nc, tc, cfg, TT):
    N, NPC, W, GPC, TS = cfg.N, cfg.NPC, cfg.W, cfg.GPC, cfg.TS
    D, ND, GD = cfg.D, cfg.ND, cfg.GD
    KS, OFFS, KM = cfg.KS, cfg.OFFS, cfg.KM
    TW = D + 8
    RG = [list(range(cfg.NC))]

    import contextlib
    ctx = contextlib.ExitStack()
    with ctx:
        pers = ctx.enter_context(tc.tile_pool(name="pers", bufs=1))
        dpool = ctx.enter_context(tc.tile_pool(name="dram", bufs=1, space="DRAM"))

        ident_sb = pers.tile([P, P], BF, tag="ident")
        nc.sync.dma_start(ident_sb[:], TT["ident_in"][:])
        ones_sb = pers.tile([1, P], BF, tag="ones")
        nc.sync.dma_start(ones_sb[:], TT["ones_in"][:])
        iota_sb = pers.tile([P, P], BF, tag="iota")
        nc.sync.dma_start(iota_sb[:], TT["iota_in"][:])
        iotac_sb = pers.tile([P, 1], BF, tag="iotac")
        nc.sync.dma_start(iotac_sb[:], TT["iotac_in"][:])

        # layer weights resident in SBUF
        W0_sb = pers.tile([ND, D], BF, tag="W0")
        nc.sync.dma_start(W0_sb[:], TT["W0"][:])
        W_sb, asrc_b, adst_b, bias_b = [None], [], [], []
        for l in range(1, 4):
            wt = pers.tile([P, 2 * D], BF, tag=f"Wt{l}")
            for c2 in range(2):
                nc.sync.dma_start(wt[:, c2 * D:(c2 + 1) * D],
                                  TT[f"W{l}"][c2 * P:(c2 + 1) * P, :])
            W_sb.append(wt)
        for l in range(4):
            a1 = pers.tile([P, D], BF, tag=f"as{l}")
            nc.sync.dma_start(a1[:], TT[f"asrcf{l}"][:])
            asrc_b.append(a1)
            a2 = pers.tile([P, D], BF, tag=f"ad{l}")
            nc.sync.dma_start(a2[:], TT[f"adstf{l}"][:])
            adst_b.append(a2)
            a3 = pers.tile([P, D], BF, tag=f"bi{l}")
            nc.sync.dma_start(a3[:], TT[f"bias{l}"][:])
            bias_b.append(a3)

        adst_buf = [pers.tile([P, W * 8], BF, tag=f"adstA{i}", name=f"adstA{i}")
                    for i in range(2)]
        nc.vector.memset(adst_buf[0][:], 0.0)
        nc.vector.memset(adst_buf[1][:], 0.0)

        # DRAM scratch
        lin_local = dpool.tile([NPC, TW], BF, tag="lin_local")
        tables = [dpool.tile([N, TW], BF, tag=f"table{li}", name=f"table{li}",
                             addr_space="Shared") for li in range(4)]
        hres = [dpool.tile([NPC, D], BF, tag=f"hres{i}", name=f"hres{i}")
                for i in range(2)]
        hfin_local = dpool.tile([NPC, D], BF, tag="hfin_local")
        hfin_table = dpool.tile([N, D], BF, tag="hfin_table", addr_space="Shared")
        M_dram = dpool.tile([P, cfg.SK, P], F8, tag="M_dram")
        MT_dram = dpool.tile([P, cfg.SK, P], F8, tag="MT_dram")

        with tc.tile_pool(name="win", bufs=2) as win, \
             tc.tile_pool(name="psA", bufs=2, space="PSUM") as psA, \
             tc.tile_pool(name="psN", bufs=2, space="PSUM") as psN, \
             tc.tile_pool(name="psL", bufs=2, space="PSUM") as psL, \
             tc.tile_pool(name="psT", bufs=2, space="PSUM") as psT:

            def phaseA_tail(nl, w, lin_ps, cnt):
                """From lin_ps (PSUM f32 [P,256], rows :cnt valid) of layer nl,
                compute lin_sb + a_src cols, a_dst col block; store lin_local."""
                n0 = w * P
                Hn = HS[nl]
                lin_sb = win.tile([P, TW], BF, tag="lin_sb")
                nc.vector.tensor_copy(lin_sb[:cnt, 0:D], lin_ps[:cnt])
                tmp = win.tile([P, D], BF, tag="tmpA")
                nc.vector.tensor_tensor(out=tmp[:cnt], in0=lin_sb[:cnt, 0:D],
                                        in1=asrc_b[nl][:cnt], op=OP.mult)
                ad = adst_buf[nl % 2]
                with nc.allow_low_precision(reason="a_src/a_dst are tiny sums"):
                    if Hn == 1:
                        nc.vector.reduce_sum(out=lin_sb[:cnt, D:D + 1],
                                             in_=tmp[:cnt],
                                             axis=mybir.AxisListType.X)
                    else:
                        nc.vector.reduce_sum(
                            out=lin_sb[:cnt, D:D + Hn],
                            in_=tmp[:cnt].rearrange("p (c h) -> p h c", c=D // Hn),
                            axis=mybir.AxisListType.X)
                    nc.vector.tensor_tensor(out=tmp[:cnt], in0=lin_sb[:cnt, 0:D],
                                            in1=adst_b[nl][:cnt], op=OP.mult)
                    if Hn == 1:
                        nc.vector.reduce_sum(out=ad[:cnt, w * 8:w * 8 + 1],
                                             in_=tmp[:cnt],
                                             axis=mybir.AxisListType.X)
                    else:
                        nc.vector.reduce_sum(
                            out=ad[:cnt, w * 8:w * 8 + Hn],
                            in_=tmp[:cnt].rearrange("p (c h) -> p h c", c=D // Hn),
                            axis=mybir.AxisListType.X)
                nc.sync.dma_start(lin_local[n0:n0 + cnt, 0:D + Hn],
                                  lin_sb[:cnt, 0:D + Hn])

            # ---------------- prologue: phase A of layer 0 ----------------
            for w in range(W):
                n0 = w * P
                cnt = min(P, NPC - n0)
                xTw = win.tile([ND, P], BF, tag="xTw")
                nc.sync.dma_start(xTw[:, :cnt], TT["xT"][:, n0:n0 + cnt])
                lin_ps = psA.tile([P, D], F32, tag="lin_ps")
                nc.tensor.matmul(lin_ps[:cnt], lhsT=xTw[:, :cnt], rhs=W0_sb[:],
                                 start=True, stop=True)
                phaseA_tail(0, w, lin_ps, cnt)

            # ------- one-time build of fp8 indicator matrices in DRAM -------
            # (overlaps with the first AllGather)
            for w in range(W):
                K = KS[w]
                E0 = int(OFFS[w])
                dcol_sb = win.tile([P, KM], BF, tag="dcol_sb")
                nc.sync.dma_start(dcol_sb[:, :K], TT["dcol_in"][:, E0:E0 + K])
                M_b = win.tile([P, KM, P], F8, tag="M_b")
                nc.vector.tensor_tensor(
                    out=M_b[:, :K, :],
                    in0=dcol_sb[:, :K, None].to_broadcast([P, K, P]),
                    in1=iota_sb[:, None, :].to_broadcast([P, K, P]),
                    op=OP.is_equal)
                nc.sync.dma_start(M_dram[:, E0:E0 + K, :], M_b[:, :K, :])
                drow_full = win.tile([P, KM * P], BF, tag="drow_full")
                nc.sync.dma_start(
                    drow_full[:, :K * P],
                    TT["drow_in"][:, E0 * P:(E0 + K) * P].to_broadcast([P, K * P]))
                MT_b = win.tile([P, KM, P], F8, tag="MT_b")
                nc.vector.tensor_tensor(
                    out=MT_b[:, :K, :].rearrange("p k e -> p (k e)"),
                    in0=iotac_sb[:].to_broadcast([P, K * P]),
                    in1=drow_full[:, :K * P],
                    op=OP.is_equal)
                nc.sync.dma_start(MT_dram[:, E0:E0 + K, :], MT_b[:, :K, :])

            # ---------------- GAT layers ----------------
            for li in range(4):
                H = HS[li]
                DN = TW if H == 8 else D + 1
                if cfg.FAKE_AG:
                    nc.sync.dma_start(tables[li][0:NPC, :], lin_local[:])
                else:
                    nc.gpsimd.collective_compute(
                        "AllGather", OP.bypass, replica_groups=RG,
                        ins=[lin_local[:]], outs=[tables[li][:]])
                adst_cur = adst_buf[li % 2]

                for w in range(W):
                    n0 = w * P
                    cnt = min(P, NPC - n0)
                    K = KS[w]
                    E0 = int(OFFS[w])

                    idx_sb = win.tile([P, KM], I32, tag="idx_sb")
                    nc.sync.dma_start(idx_sb[:, :K], TT["idx_in"][:, E0:E0 + K])
                    M_sb = win.tile([P, KM, P], F8, tag="M_sb")
                    MT_sb = win.tile([P, KM, P], F8, tag="MT_sb")
                    if not cfg.SKIP_MMT:
                        nc.sync.dma_start(M_sb[:, :K, :], M_dram[:, E0:E0 + K, :])
                        nc.sync.dma_start(MT_sb[:, :K, :], MT_dram[:, E0:E0 + K, :])
                    aE_sb = win.tile([P, KM, 8], BF, tag="aE_sb")
                    nc.sync.dma_start(aE_sb[:, :K, :H], TT[f"aE{li}"][:, E0:E0 + K, :])

                    lin_g = win.tile([P, KM, TW], BF, tag="lin_g")
                    if cfg.SKIP_GATHER:
                        nc.vector.memset(lin_g[:], 0.0)
                    else:
                        for k in range(K):
                            nc.gpsimd.indirect_dma_start(
                                out=lin_g[:, k, :], out_offset=None,
                                in_=tables[li][:],
                                in_offset=bass.IndirectOffsetOnAxis(
                                    ap=idx_sb[:, k:k + 1], axis=0))

                    # alpha = lrelu(a_src + a_dst + a_edge, 0.2); ex = exp
                    al_ps = psL.tile([P, KM * 8], F32, tag="al_ps")
                    for k in range(K):
                        nc.tensor.matmul(al_ps[:, k * 8:k * 8 + H],
                                         lhsT=MT_sb[:, k, :],
                                         rhs=adst_cur[:, w * 8:w * 8 + H],
                                         start=True, stop=True)
                    al_sb = win.tile([P, KM, 8], BF, tag="al_sb")
                    nc.vector.tensor_tensor(
                        out=al_sb[:, :K, :H],
                        in0=al_ps[:].rearrange("p (k h) -> p k h", k=KM)[:, :K, :H],
                        in1=aE_sb[:, :K, :H], op=OP.add)
                    nc.vector.tensor_tensor(
                        out=al_sb[:, :K, :H], in0=al_sb[:, :K, :H],
                        in1=lin_g[:, :K, D:D + H], op=OP.add)
                    lr_sb = win.tile([P, KM, 8], BF, tag="lr_sb")
                    nc.vector.tensor_scalar_mul(lr_sb[:, :K, :H],
                                                al_sb[:, :K, :H], 0.2)
                    nc.vector.tensor_tensor(out=lr_sb[:, :K, :H],
                                            in0=lr_sb[:, :K, :H],
                                            in1=al_sb[:, :K, :H], op=OP.max)
                    ex_sb = win.tile([P, KM, 8], BF, tag="ex_sb")
                    nc.scalar.activation(ex_sb[:, :K, :H], lr_sb[:, :K, :H], AF.Exp)
                    if H == 1:
                        exu = win.tile([P, KM, 8], BF, tag="exu")
                        nc.vector.tensor_copy(
                            exu[:, :K, :],
                            ex_sb[:, :K, 0:1].to_broadcast([P, K, 8]))
                    else:
                        exu = ex_sb

                    # wfex = lin_g * ex (broadcast over c; h innermost -> 2x)
                    wfex = win.tile([P, KM, TW], BF, tag="wfex")
                    if not cfg.SKIP_WFEX:
                        nc.vector.tensor_tensor(
                            out=wfex[:, :K, 0:D].rearrange("p k (c h) -> p k c h", c=32),
                            in0=lin_g[:, :K, 0:D].rearrange("p k (c h) -> p k c h", c=32),
                            in1=exu[:, :K, None, :].to_broadcast([P, K, 32, 8]),
                            op=OP.mult)
                    if H == 8:
                        nc.vector.tensor_copy(wfex[:, :K, D:D + 8], exu[:, :K, :])
                    else:
                        nc.vector.tensor_copy(wfex[:, :K, D:D + 1], exu[:, :K, 0:1])

                    nu_ps = psN.tile([P, TW], F32, tag="nu_ps")
                    if cfg.SKIP_NU:
                        nc.tensor.matmul(nu_ps[:, 0:DN], lhsT=M_sb[:, 0, :],
                                         rhs=wfex[:, 0, 0:DN],
                                         start=True, stop=True)
                    else:
                        for k in range(K):
                            nc.tensor.matmul(nu_ps[:, 0:DN], lhsT=M_sb[:, k, :],
                                             rhs=wfex[:, k, 0:DN],
                                             start=(k == 0), stop=(k == K - 1))

                    # normalize, bias, ELU, residual
                    den = win.tile([P, 8], F32, tag="den")
                    nc.vector.tensor_scalar_add(den[:cnt, :H],
                                                nu_ps[:cnt, D:D + H], 1e-16)
                    rec = win.tile([P, 8], F32, tag="rec")
                    nc.vector.reciprocal(rec[:cnt, :H], den[:cnt, :H])
                    outw = win.tile([P, D], BF, tag="outw")
                    if H == 8:
                        nc.vector.tensor_tensor(
                            out=outw[:cnt].rearrange("p (c h) -> p c h", c=32),
                            in0=nu_ps[:cnt, 0:D].rearrange("p (c h) -> p c h", c=32),
                            in1=rec[:cnt, None, :H].to_broadcast([cnt, 32, 8]),
                            op=OP.mult)
                    else:
                        nc.vector.tensor_tensor(
                            out=outw[:cnt], in0=nu_ps[:cnt, 0:D],
                            in1=rec[:cnt, 0:1].to_broadcast([cnt, D]),
                            op=OP.mult)
                    nc.vector.tensor_tensor(out=outw[:cnt], in0=outw[:cnt],
                                            in1=bias_b[li][:cnt], op=OP.add)
                    # ELU = relu(x) + exp(min(x,0)) - 1
                    tmin = win.tile([P, D], BF, tag="tmin")
                    nc.vector.tensor_scalar_min(tmin[:cnt], outw[:cnt], 0.0)
                    nc.scalar.activation(tmin[:cnt], tmin[:cnt], AF.Exp)
                    trel = win.tile([P, D], BF, tag="trel")
                    nc.vector.tensor_scalar_max(trel[:cnt], outw[:cnt], 0.0)
                    hn = win.tile([P, D], BF, tag="hn")
                    if cnt < P:
                        nc.vector.memset(hn[:], 0.0)
                    nc.vector.tensor_tensor(out=hn[:cnt], in0=tmin[:cnt],
                                            in1=trel[:cnt], op=OP.add)
                    nc.vector.tensor_scalar_add(hn[:cnt], hn[:cnt], -1.0)
                    if li > 0:
                        hp = win.tile([P, D], BF, tag="hp")
                        nc.sync.dma_start(hp[:cnt], hres[(li - 1) % 2][n0:n0 + cnt])
                        if li == 3:
                            # hp is stored (c-major) space; hn is natural space
                            nc.vector.tensor_tensor(
                                out=hn[:cnt].rearrange("p (h c) -> p h c", h=8),
                                in0=hn[:cnt].rearrange("p (h c) -> p h c", h=8),
                                in1=hp[:cnt].rearrange("p (c h) -> p h c", c=32),
                                op=OP.add)
                        else:
                            nc.vector.tensor_tensor(out=hn[:cnt], in0=hn[:cnt],
                                                    in1=hp[:cnt], op=OP.add)
                    if li == 3:
                        nc.sync.dma_start(hfin_local[n0:n0 + cnt], hn[:cnt])
                        continue
                    nc.sync.dma_start(hres[li % 2][n0:n0 + cnt], hn[:cnt])

                    # fused phase A of layer li+1
                    nl = li + 1
                    trc = win.tile([P, 2 * P], BF, tag="trc")
                    for c2 in range(2):
                        tr_ps = psT.tile([P, P], BF, tag="tr_ps")
                        nc.tensor.transpose(tr_ps[:], hn[:, c2 * P:(c2 + 1) * P],
                                            ident_sb[:])
                        nc.vector.tensor_copy(trc[:, c2 * P:(c2 + 1) * P], tr_ps[:])
                    lin_ps = psA.tile([P, D], F32, tag="lin_ps")
                    for c2 in range(2):
                        nc.tensor.matmul(
                            lin_ps[:cnt],
                            lhsT=trc[:, c2 * P:c2 * P + cnt],
                            rhs=W_sb[nl][:, c2 * D:(c2 + 1) * D],
                            start=(c2 == 0), stop=(c2 == 1))
                    phaseA_tail(nl, w, lin_ps, cnt)

            # final AllGather of node features for set2set
            if cfg.FAKE_AG:
                nc.sync.dma_start(hfin_table[0:NPC, :], hfin_local[:])
            else:
                nc.gpsimd.collective_compute(
                    "AllGather", OP.bypass, replica_groups=RG,
                    ins=[hfin_local[:]], outs=[hfin_table[:]])

        build_s2s(nc, tc, cfg, TT, pers, hfin_table, ident_sb, ones_sb)


def build_s2s(nc, tc, cfg, TT, pers, hfin_table, ident_sb, ones_sb):
    NPC, GPC, TS = cfg.NPC, cfg.GPC, cfg.TS
    D, GD = cfg.D, cfg.GD
    GG = GPC
    STEPS = cfg.S2S_STEPS

    with tc.tile_pool(name="s2s", bufs=1) as sp, \
         tc.tile_pool(name="ps2", bufs=1, space="PSUM") as ps2:
        s2s_idx = sp.tile([P, TS], I32, tag="s2s_idx")
        nc.sync.dma_start(s2s_idx[:], TT["s2s_idx_in"][:])
        xn = sp.tile([P, TS, D], BF, tag="xn")
        for t in range(TS):
            nc.gpsimd.indirect_dma_start(
                out=xn[:, t, :], out_offset=None, in_=hfin_table[:],
                in_offset=bass.IndirectOffsetOnAxis(ap=s2s_idx[:, t:t + 1], axis=0))
        Mb = sp.tile([P, TS * GG], F8, tag="Mb")
        nc.sync.dma_start(Mb[:], TT["Mb_in"][:])
        MbT = sp.tile([GG, TS * P], F8, tag="MbT")
        nc.sync.dma_start(MbT[:], TT["MbT_in"][:])

        wih = sp.tile([P, 4 * 4 * D], BF, tag="wih")
        for c2 in range(4):
            nc.sync.dma_start(wih[:, c2 * 4 * D:(c2 + 1) * 4 * D],
                              TT["WihT"][c2 * P:(c2 + 1) * P, :])
        whh = sp.tile([P, 2 * 4 * D], BF, tag="whh")
        for c2 in range(2):
            nc.sync.dma_start(whh[:, c2 * 4 * D:(c2 + 1) * 4 * D],
                              TT["WhhT"][c2 * P:(c2 + 1) * P, :])
        s2sb = sp.tile([1, 4 * D], BF, tag="s2sb")
        nc.sync.dma_start(s2sb[:], TT["s2s_bias"][:])

        qT = [sp.tile([P, GG], BF, tag=f"qT{c2}", name=f"qT{c2}") for c2 in range(4)]
        c_st = sp.tile([GG, D], F32, tag="c_st")
        for t_ in qT:
            nc.vector.memset(t_[:], 0.0)
        nc.vector.memset(c_st[:], 0.0)

        gact = [AF.Sigmoid, AF.Sigmoid, AF.Tanh, AF.Sigmoid]  # i, f, g, o
        for step in range(STEPS):
            gs = []
            for g in range(4):
                g_ps = ps2.tile([GG, D], F32, tag="psY")
                nc.tensor.matmul(g_ps[:], lhsT=ones_sb[:, 0:GG],
                                 rhs=s2sb[:, g * D:(g + 1) * D],
                                 start=True, stop=False)
                for c2 in range(4):
                    nc.tensor.matmul(
                        g_ps[:], lhsT=qT[c2][:],
                        rhs=wih[:, c2 * 4 * D + g * D: c2 * 4 * D + (g + 1) * D],
                        start=False, stop=False)
                for c2 in range(2):
                    nc.tensor.matmul(
                        g_ps[:], lhsT=qT[c2][:],
                        rhs=whh[:, c2 * 4 * D + g * D: c2 * 4 * D + (g + 1) * D],
                        start=False, stop=(c2 == 1))
                g_sb = sp.tile([GG, D], F32, tag=f"g_sb{g}")
                nc.scalar.activation(g_sb[:], g_ps[:], gact[g])
                gs.append(g_sb)
            t1 = sp.tile([GG, D], F32, tag="t1")
            nc.vector.tensor_tensor(out=t1[:], in0=gs[0][:], in1=gs[2][:], op=OP.mult)
            nc.vector.tensor_tensor(out=c_st[:], in0=gs[1][:], in1=c_st[:], op=OP.mult)
            nc.vector.tensor_tensor(out=c_st[:], in0=c_st[:], in1=t1[:], op=OP.add)
            tc_sb = sp.tile([GG, D], F32, tag="tc_sb")
            nc.scalar.activation(tc_sb[:], c_st[:], AF.Tanh)
            h_l = sp.tile([GG, D], BF, tag="h_l")
            nc.vector.tensor_tensor(out=h_l[:], in0=gs[3][:], in1=tc_sb[:], op=OP.mult)

            # attention over nodes: e = <xn, h[batch]>, softmax per graph
            e_all = sp.tile([P, TS], F32, tag="e_all")
            escr = sp.tile([P, D], BF, tag="escr")
            for t in range(TS):
                he_ps = ps2.tile([P, D], F32, tag="psH")
                nc.tensor.matmul(he_ps[:], lhsT=MbT[:, t * P:(t + 1) * P],
                                 rhs=h_l[:], start=True, stop=True)
                nc.vector.tensor_tensor(out=escr[:], in0=xn[:, t, :],
                                        in1=he_ps[:], op=OP.mult)
                nc.vector.reduce_sum(out=e_all[:, t:t + 1], in_=escr[:],
                                     axis=mybir.AxisListType.X)
            eb = sp.tile([P, TS], BF, tag="eb")
            nc.scalar.activation(eb[:], e_all[:], AF.Exp)
            r_ps = ps2.tile([GG, D + 1], F32, tag="psR")
            for t in range(TS):
                wxex = sp.tile([P, D + 1], BF, tag="wxex")
                nc.vector.tensor_tensor(
                    out=wxex[:, 0:D], in0=xn[:, t, :],
                    in1=eb[:, t:t + 1].to_broadcast([P, D]), op=OP.mult)
                nc.vector.tensor_copy(wxex[:, D:D + 1], eb[:, t:t + 1])
                nc.tensor.matmul(r_ps[:], lhsT=Mb[:, t * GG:(t + 1) * GG],
                                 rhs=wxex[:], start=(t == 0), stop=(t == TS - 1))
            den = sp.tile([GG, 1], F32, tag="s2s_den")
            nc.vector.tensor_scalar_add(den[:], r_ps[:, D:D + 1], 1e-16)
            rec = sp.tile([GG, 1], F32, tag="s2s_rec")
            nc.vector.reciprocal(rec[:], den[:])
            r_sb = sp.tile([GG, D], BF, tag="r_sb")
            nc.vector.tensor_tensor(out=r_sb[:], in0=r_ps[:, 0:D],
                                    in1=rec[:].to_broadcast([GG, D]), op=OP.mult)
            for c2 in range(2):
                tr_ps = ps2.tile([P, GG], BF, tag="psX")
                nc.tensor.transpose(tr_ps[:], h_l[:, c2 * P:(c2 + 1) * P],
                                    ident_sb[:GG, :GG])
                nc.vector.tensor_copy(qT[c2][:], tr_ps[:])
                tr_ps2 = ps2.tile([P, GG], BF, tag="psX")
                nc.tensor.transpose(tr_ps2[:], r_sb[:, c2 * P:(c2 + 1) * P],
                                    ident_sb[:GG, :GG])
                nc.vector.tensor_copy(qT[2 + c2][:], tr_ps2[:])

        # ---------------- MLP head ----------------
        gfT_sb = sp.tile([GD, GG], BF, tag="gfT_sb")
        nc.sync.dma_start(gfT_sb[:], TT["gfT_in"][:])
        p1w_sb = sp.tile([P, 4 * D], BF, tag="p1w_sb")
        for c2 in range(4):
            nc.sync.dma_start(p1w_sb[:, c2 * D:(c2 + 1) * D],
                              TT["p1W"][c2 * P:(c2 + 1) * P, :])
        p1wg_sb = sp.tile([GD, D], BF, tag="p1wg_sb")
        nc.sync.dma_start(p1wg_sb[:], TT["p1W"][4 * P:4 * P + GD, :])
        p1b_sb = sp.tile([1, D], BF, tag="p1b_sb")
        nc.sync.dma_start(p1b_sb[:], TT["p1b"][:])
        z1_ps = ps2.tile([GG, D], F32, tag="psY")
        nc.tensor.matmul(z1_ps[:], lhsT=ones_sb[:, 0:GG], rhs=p1b_sb[:],
                         start=True, stop=False)
        for c2 in range(4):
            nc.tensor.matmul(z1_ps[:], lhsT=qT[c2][:],
                             rhs=p1w_sb[:, c2 * D:(c2 + 1) * D],
                             start=False, stop=False)
        nc.tensor.matmul(z1_ps[:], lhsT=gfT_sb[:], rhs=p1wg_sb[:],
                         start=False, stop=True)
        z1 = sp.tile([GG, D], BF, tag="z1")
        nc.scalar.activation(z1[:], z1_ps[:], AF.Relu)

        p2w_sb = sp.tile([P, 2 * (D // 2)], BF, tag="p2w_sb")
        for c2 in range(2):
            nc.sync.dma_start(p2w_sb[:, c2 * (D // 2):(c2 + 1) * (D // 2)],
                              TT["p2W"][c2 * P:(c2 + 1) * P, :])
        p2b_sb = sp.tile([1, D // 2], BF, tag="p2b_sb")
        nc.sync.dma_start(p2b_sb[:], TT["p2b"][:])
        z2_ps = ps2.tile([GG, D // 2], F32, tag="psY")
        nc.tensor.matmul(z2_ps[:], lhsT=ones_sb[:, 0:GG], rhs=p2b_sb[:],
                         start=True, stop=False)
        for c2 in range(2):
            z1T_ps = ps2.tile([P, GG], BF, tag="psX")
            nc.tensor.transpose(z1T_ps[:], z1[:, c2 * P:(c2 + 1) * P],
                                ident_sb[:GG, :GG])
            z1T = sp.tile([P, GG], BF, tag="z1T")
            nc.vector.tensor_copy(z1T[:], z1T_ps[:])
            nc.tensor.matmul(z2_ps[:], lhsT=z1T[:],
                             rhs=p2w_sb[:, c2 * (D // 2):(c2 + 1) * (D // 2)],
                             start=False, stop=(c2 == 1))
        z2 = sp.tile([GG, D // 2], BF, tag="z2")
        nc.scalar.activation(z2[:], z2_ps[:], AF.Relu)

        p3w_sb = sp.tile([D // 2, 5], BF, tag="p3w_sb")
        nc.sync.dma_start(p3w_sb[:], TT["p3W"][:])
        p3b_sb = sp.tile([1, 5], BF, tag="p3b_sb")
        nc.sync.dma_start(p3b_sb[:], TT["p3b"][:])
        z2T_ps = ps2.tile([P, GG], BF, tag="psX")
        nc.tensor.transpose(z2T_ps[:], z2[:], ident_sb[:GG, :GG])
        z2T = sp.tile([P, GG], BF, tag="z2T")
        nc.vector.tensor_copy(z2T[:], z2T_ps[:])
        o_ps = ps2.tile([GG, 5], F32, tag="psY")
        nc.tensor.matmul(o_ps[:], lhsT=ones_sb[:, 0:GG], rhs=p3b_sb[:],
                         start=True, stop=False)
        nc.tensor.matmul(o_ps[:], lhsT=z2T[:], rhs=p3w_sb[:],
                         start=False, stop=True)
        o_sb = sp.tile([GG, 5], F32, tag="o_sb")
        nc.vector.tensor_copy(o_sb[:], o_ps[:])
        nc.sync.dma_start(TT["out"][:], o_sb[:cfg.GPC])


def run_config(inputs, cfg):
    in_maps = host_prep(inputs, cfg)
    nc = build_kernel_with_maps(cfg)
    res = run_bass_kernel_spmd(nc, in_maps, core_ids=list(range(cfg.NC)))
    out = np.concatenate([res.results[c]["out"] for c in range(cfg.NC)], axis=0)
    return out.astype(np.float32)


def build_kernel_with_maps(cfg):
    return build_kernel(cfg)


def kernel(**inputs):
    return run_config(inputs, CFG.derive())


# revision 33
# speedup vs baseline: 1.2097x; 1.2097x over previous
"""Trainium2 Bass kernel for AdvancedGATModel (4-layer edge-featured GAT +
Set2Set pooling + MLP head), sharded across 8 NeuronCores.

Sharding: nodes are split into 8 contiguous slices (6250 each); each core owns
the edges whose *destination* lands in its slice (plus self-loops), so segment
softmax and the scatter-add aggregation are core-local.  Per layer each core
computes the linear transform of its node slice, all cores AllGather the
transformed features (bf16) into a replicated table, and each core gathers its
edges' source rows via indirect DMA.  Segment softmax/weighted-sum run as
dense 128-edge-tile matmuls against host-precomputed fp8 indicator matrices
(edges x window-nodes and its transpose).  Per-edge "a_edge" attention terms
are host-precomputed per layer.  Features use a head-minor (c-major) column
layout so the per-edge softmax scaling runs in the DVE 2x packed mode.  The
linear transform of layer l+1 is fused into layer l's per-window epilogue, so
no transposed-feature round trip through DRAM is needed.  Set2Set is sharded
by graph (64 graphs/core; graphs never straddle cores) and runs in
bf16/f32r, and the [64,5] head outputs are concatenated on host.

The program is identical on all 8 cores (SPMD); only input *data* differs.
All shapes below are hardcoded for the grading problem.

This version gathers edge-source rows with gpsimd.dma_gather (Q7 `mlp`
library), chunked at LCH=8 tiles = 1024 idxs per instruction: ~3 gather
instructions per window instead of 18 indirect_dma_starts (whose ~960ns
per-instruction Q7 descriptor generation cost ~3.4ms total).  Hard-won HW
facts: dma_gather idx count <=1024 works, 2048+ fails (INTERNAL/hang);
int16 idx wrapped [16, n/16] then tiled to [128, n/16]; nonzero in_ap base
offsets (tables[li][25600:]) are fine; rows must be a multiple of 256B
(hence TP=384 padded bf16 elems); a [P,K]-offset indirect_dma_start hangs
HW (sim-only).  Measured (median-diff vs null kernel, device-resident
inputs): ~4.2ms vs ~5.2ms for the per-tile indirect version in the same
measurement window (method drift across windows is +-15%).  Remaining
levers: AllGathers ~1.2ms total (5 x ~245us) -- chunk them and issue
mid-loop for overlap (~50-200us/layer given the ~15us + 40-110GB/s
collective cost curve); probe dma_gather between 1024 and 2048 idxs; and
the +45% padded-row gather traffic could drop by gathering elem_size=512B
(h only) if a_src-per-edge finds a cheap home.
"""

import numpy as np

import concourse.bass as bass
import concourse.bacc as bacc
import concourse.tile as tile
import concourse.mybir as mybir
from concourse.bass_utils import run_bass_kernel_spmd

F32 = mybir.dt.float32
F32R = mybir.dt.float32r
BF = mybir.dt.bfloat16
F8 = mybir.dt.float8e4
I32 = mybir.dt.int32
AF = mybir.ActivationFunctionType
OP = mybir.AluOpType
P = 128

NPBF = mybir.dt.np(BF)
NPF8 = mybir.dt.np(F8)

HS = [8, 8, 8, 1]          # heads per layer
# stored feature j = c*8+h  <->  reference feature h*32+c  (layers 0-2)
PSTORE = np.array([h * 32 + c for c in range(32) for h in range(8)], np.int64)


class CFG:
    # full problem; small-mode tests override these
    N = 50000          # nodes
    E = 800000         # edges (before self loops)
    G = 512            # graphs
    ND = 14            # node feat dim
    ED = 4             # edge feat dim
    GD = 13            # global feat dim
    D = 256            # hidden
    H = 8              # heads
    C = 32             # per-head channels
    NC = 8             # cores
    NPC = N // NC      # nodes per core = 6250
    W = 49             # node windows per core (ceil(NPC/128))
    GPC = G // NC      # graphs per core = 64
    TS = 51            # set2set node tiles per core (capacity TS*128 nodes)
    S2S_STEPS = 3
    LAYERS = 4
    FAKE_AG = False    # replace AllGathers with local copies (debug only)
    # timing-attribution probes (wrong outputs; timing only)
    SKIP_GATHER = False
    SKIP_MMT = False
    SKIP_WFEX = False
    SKIP_NU = False
    # set by host_prep:
    KS = None          # per-window edge tile counts (common across cores)
    SK = None          # sum(KS)
    KM = None          # max(KS)
    OFFS = None        # cumulative offsets

    @classmethod
    def derive(cls):
        cls.NPC = cls.N // cls.NC
        cls.GPC = cls.G // cls.NC
        cls.W = -(-cls.NPC // P)
        return cls


def make_small_cfg():
    class Small(CFG):
        N = 2048
        E = 8192
        G = 64
        W = 2
        TS = 3
    return Small.derive()


# ------------------------------------------------------------------
# host-side preprocessing
# ------------------------------------------------------------------

def host_prep(inp, cfg):
    """Build per-core input maps from the full input dict."""
    N, E, G = cfg.N, cfg.E, cfg.G
    NC, NPC, W, GPC, TS = cfg.NC, cfg.NPC, cfg.W, cfg.GPC, cfg.TS
    D, ED, GD = cfg.D, cfg.ED, cfg.GD

    src = np.asarray(inp["edge_index"][0])
    dst = np.asarray(inp["edge_index"][1])
    ea = np.asarray(inp["edge_attr"], dtype=np.float32)
    batch = np.asarray(inp["batch_idx"])
    x = np.asarray(inp["x"], dtype=np.float32)

    # self-loop attr = mean incoming edge attr (0 for isolated nodes)
    deg = np.bincount(dst, minlength=N).astype(np.float32)
    loop = np.zeros((N, ED), np.float32)
    for j in range(ED):
        loop[:, j] = np.bincount(dst, weights=ea[:, j], minlength=N)
    loop /= np.maximum(deg, 1.0)[:, None]

    src2 = np.concatenate([src, np.arange(N, dtype=np.int64)])
    dst2 = np.concatenate([dst, np.arange(N, dtype=np.int64)])
    ea2 = np.concatenate([ea, loop], axis=0).astype(np.float32)

    order = np.argsort(dst2, kind="stable")
    s_src = src2[order]
    s_dst = dst2[order]
    s_ea = ea2[order]

    # per-layer weights: head-minor (c-major) permutation of 256-wide dims
    Wms, asrcs, adsts, biases, aEs = [], [], [], [], []
    for i in range(4):
        Wm = np.asarray(inp[f"g{i}_W"], np.float32)          # [din, H*C]
        We = np.asarray(inp[f"g{i}_We"], np.float32)         # [ED, H*C]
        asrc = np.asarray(inp[f"g{i}_asrc"], np.float32)     # [h, c]
        adst = np.asarray(inp[f"g{i}_adst"], np.float32)
        aedge = np.asarray(inp[f"g{i}_aedge"], np.float32)
        b = np.asarray(inp[f"g{i}_b"], np.float32)
        h, c = asrc.shape
        if i > 0:
            Wm = Wm[PSTORE, :]          # input comes in stored (c-major) space
        af, df, bf = asrc.reshape(-1), adst.reshape(-1), b
        if i < 3:
            Wm = Wm[:, PSTORE]          # output in stored space
            af, df, bf = af[PSTORE], df[PSTORE], bf[PSTORE]
        Wms.append(Wm)
        asrcs.append(np.broadcast_to(af, (P, D)).copy())
        adsts.append(np.broadcast_to(df, (P, D)).copy())
        biases.append(np.broadcast_to(bf, (P, D)).copy())
        # per-edge attention term: aE[e, h] = sum_d ea2[e,d] * M_ae[d,h]
        M_ae = (We.reshape(ED, h, c) * aedge[None]).sum(-1)  # [ED, h]
        aEs.append((s_ea @ M_ae).astype(np.float32))         # [E2 sorted, h]

    # -------- per-core edge ranges and common per-window tile counts -----
    e_bounds = [(np.searchsorted(s_dst, c2 * NPC), np.searchsorted(s_dst, (c2 + 1) * NPC))
                for c2 in range(NC)]
    cnts = np.zeros((NC, W), np.int64)
    for ci in range(NC):
        e0, e1 = e_bounds[ci]
        cd = s_dst[e0:e1] - ci * NPC
        for w in range(W):
            lo, hi = w * P, min((w + 1) * P, NPC)
            cnts[ci, w] = np.searchsorted(cd, hi) - np.searchsorted(cd, lo)
    KS = [int(-(-cnts[:, w].max() // P)) for w in range(W)]
    KS = [max(k, 1) for k in KS]
    OFFS = np.concatenate([[0], np.cumsum(KS)]).astype(np.int64)
    SK = int(OFFS[-1])
    cfg.KS, cfg.SK, cfg.KM, cfg.OFFS = KS, SK, max(KS), OFFS

    # graph ranges per core for set2set (graphs never straddle cores)
    gbound = np.searchsorted(batch, np.arange(G + 1))

    in_maps = []
    for ci in range(NC):
        n0 = ci * NPC
        e0, e1 = e_bounds[ci]
        cs, cd = s_src[e0:e1], s_dst[e0:e1] - n0
        caE = [aE[e0:e1] for aE in aEs]

        idx_arr = np.zeros((P, SK), np.int32)
        dcol_arr = np.full((P, SK), 255.0, NPBF)
        drow_arr = np.full((1, SK * P), 255.0, NPBF)
        aE_arr = [np.zeros((P, SK, HS[l]), NPBF) for l in range(4)]
        for w in range(W):
            lo, hi = w * P, min((w + 1) * P, NPC)
            a = np.searchsorted(cd, lo)
            b2 = np.searchsorted(cd, hi)
            cnt = b2 - a
            assert cnt <= KS[w] * P
            js = np.arange(cnt)
            tk = js // P + OFFS[w]
            pp = js % P
            drel = (cd[a:b2] - lo).astype(np.int64)
            idx_arr[pp, tk] = cs[a:b2]
            dcol_arr[pp, tk] = drel.astype(NPBF)
            drow_arr[0, tk * P + pp] = drel.astype(NPBF)
            for l in range(4):
                aE_arr[l][pp, tk, :] = caE[l][a:b2].astype(NPBF)

        # set2set: node range + padding for this core's graphs
        g0 = ci * GPC
        gn0, gn1 = gbound[g0], gbound[g0 + GPC]
        ncnt = gn1 - gn0
        assert ncnt <= TS * P, f"s2s overflow core {ci}: {ncnt} > {TS*P}"
        s2s_idx = np.zeros((TS * P,), np.int32)
        s2s_idx[:ncnt] = np.arange(gn0, gn1, dtype=np.int32)
        s2s_brel = np.full((TS * P,), -1, np.int64)
        s2s_brel[:ncnt] = batch[gn0:gn1] - g0
        Mb = np.zeros((TS * P, GPC), NPF8)
        ii = np.arange(ncnt)
        Mb[ii, s2s_brel[:ncnt]] = 1.0
        Mb = Mb.reshape(TS, P, GPC)

        m = dict(
            xT=np.ascontiguousarray(x[n0:n0 + NPC].T).astype(NPBF),
            idx_in=idx_arr,
            dcol_in=dcol_arr, drow_in=drow_arr,
            iota_in=np.broadcast_to(
                np.arange(P, dtype=np.float32).astype(NPBF), (P, P)).copy(),
            iotac_in=np.arange(P, dtype=np.float32).astype(NPBF).reshape(P, 1),
            ident_in=np.eye(P, dtype=NPBF),
            ones_in=np.ones((1, P), NPBF),
            s2s_idx_in=np.ascontiguousarray(s2s_idx.reshape(TS, P).T),
            Mb_in=np.ascontiguousarray(Mb.transpose(1, 0, 2).reshape(P, TS * GPC)),
            MbT_in=np.ascontiguousarray(Mb.reshape(TS * P, GPC).T),
            gfT_in=np.ascontiguousarray(
                np.asarray(inp["global_features"], np.float32)[g0:g0 + GPC].T
            ).astype(NPBF),
        )
        for l in range(4):
            m[f"W{l}"] = Wms[l].astype(NPBF)
            m[f"asrcf{l}"] = asrcs[l].astype(NPBF)
            m[f"adstf{l}"] = adsts[l].astype(NPBF)
            m[f"bias{l}"] = biases[l].astype(NPBF)
            m[f"aE{l}"] = aE_arr[l]
        m["WihT"] = np.ascontiguousarray(np.asarray(inp["s2s_Wih"], np.float32).T).astype(NPBF)
        m["WhhT"] = np.ascontiguousarray(np.asarray(inp["s2s_Whh"], np.float32).T).astype(NPBF)
        m["s2s_bias"] = (np.asarray(inp["s2s_bih"], np.float32)
                         + np.asarray(inp["s2s_bhh"], np.float32)).reshape(1, -1).astype(NPBF)
        m["p1W"] = np.asarray(inp["p1_W"], np.float32).astype(NPBF)
        m["p1b"] = np.asarray(inp["p1_b"], np.float32).reshape(1, -1).astype(NPBF)
        m["p2W"] = np.asarray(inp["p2_W"], np.float32).astype(NPBF)
        m["p2b"] = np.asarray(inp["p2_b"], np.float32).reshape(1, -1).astype(NPBF)
        m["p3W"] = np.asarray(inp["p3_W"], np.float32).astype(NPBF)
        m["p3b"] = np.asarray(inp["p3_b"], np.float32).reshape(1, -1).astype(NPBF)
        in_maps.append(m)
    return in_maps


# ------------------------------------------------------------------
# device kernel builder
# ------------------------------------------------------------------

def build_kernel(cfg):
    N, NPC, W, GPC, TS, SK = cfg.N, cfg.NPC, cfg.W, cfg.GPC, cfg.TS, cfg.SK
    D, ND, GD = cfg.D, cfg.ND, cfg.GD

    nc = bacc.Bacc("TRN2", target_bir_lowering=False, debug=False,
                   num_devices=cfg.NC)

    TT = {}

    def din(name, shape, dt, kind="ExternalInput"):
        TT[name] = nc.dram_tensor(name, shape, dt, kind=kind)

    din("xT", [ND, NPC], BF)
    din("idx_in", [P, SK], I32)
    din("dcol_in", [P, SK], BF)
    din("drow_in", [1, SK * P], BF)
    din("iota_in", [P, P], BF)
    din("iotac_in", [P, 1], BF)
    din("ident_in", [P, P], BF)
    din("ones_in", [1, P], BF)
    for l in range(4):
        d0 = ND if l == 0 else D
        din(f"W{l}", [d0, D], BF)
        din(f"asrcf{l}", [P, D], BF)
        din(f"adstf{l}", [P, D], BF)
        din(f"bias{l}", [P, D], BF)
        din(f"aE{l}", [P, SK, HS[l]], BF)
    din("s2s_idx_in", [P, TS], I32)
    din("Mb_in", [P, TS * GPC], F8)
    din("MbT_in", [GPC, TS * P], F8)
    din("gfT_in", [GD, GPC], BF)
    din("WihT", [2 * D, 4 * D], BF)
    din("WhhT", [D, 4 * D], BF)
    din("s2s_bias", [1, 4 * D], BF)
    din("p1W", [2 * D + GD, D], BF)
    din("p1b", [1, D], BF)
    din("p2W", [D, D // 2], BF)
    din("p2b", [1, D // 2], BF)
    din("p3W", [D // 2, 5], BF)
    din("p3b", [1, 5], BF)
    din("out", [GPC, 5], F32, kind="ExternalOutput")

    with tile.TileContext(nc) as tc:
        build_body(nc, tc, cfg, TT)
    nc.compile()
    return nc


def build_body(nc, tc, cfg, TT):
    N, NPC, W, GPC, TS = cfg.N, cfg.NPC, cfg.W, cfg.GPC, cfg.TS
    D, ND, GD = cfg.D, cfg.ND, cfg.GD
    KS, OFFS, KM = cfg.KS, cfg.OFFS, cfg.KM
    TW = D + 8
    RG = [list(range(cfg.NC))]

    import contextlib
    ctx = contextlib.ExitStack()
    with ctx:
        pers = ctx.enter_context(tc.tile_pool(name="pers", bufs=1))
        dpool = ctx.enter_context(tc.tile_pool(name="dram", bufs=1, space="DRAM"))

        ident_sb = pers.tile([P, P], BF, tag="ident")
        nc.sync.dma_start(ident_sb[:], TT["ident_in"][:])
        ones_sb = pers.tile([1, P], BF, tag="ones")
        nc.sync.dma_start(ones_sb[:], TT["ones_in"][:])
        iota_sb = pers.tile([P, P], BF, tag="iota")
        nc.sync.dma_start(iota_sb[:], TT["iota_in"][:])
        iotac_sb = pers.tile([P, 1], BF, tag="iotac")
        nc.sync.dma_start(iotac_sb[:], TT["iotac_in"][:])

        # layer weights resident in SBUF
        W0_sb = pers.tile([ND, D], BF, tag="W0")
        nc.sync.dma_start(W0_sb[:], TT["W0"][:])
        W_sb, asrc_b, adst_b, bias_b = [None], [], [], []
        for l in range(1, 4):
            wt = pers.tile([P, 2 * D], BF, tag=f"Wt{l}")
            for c2 in range(2):
                nc.sync.dma_start(wt[:, c2 * D:(c2 + 1) * D],
                                  TT[f"W{l}"][c2 * P:(c2 + 1) * P, :])
            W_sb.append(wt)
        for l in range(4):
            a1 = pers.tile([P, D], BF, tag=f"as{l}")
            nc.sync.dma_start(a1[:], TT[f"asrcf{l}"][:])
            asrc_b.append(a1)
            a2 = pers.tile([P, D], BF, tag=f"ad{l}")
            nc.sync.dma_start(a2[:], TT[f"adstf{l}"][:])
            adst_b.append(a2)
            a3 = pers.tile([P, D], BF, tag=f"bi{l}")
            nc.sync.dma_start(a3[:], TT[f"bias{l}"][:])
            bias_b.append(a3)

        adst_buf = [pers.tile([P, W * 8], BF, tag=f"adstA{i}", name=f"adstA{i}")
                    for i in range(2)]
        nc.vector.memset(adst_buf[0][:], 0.0)
        nc.vector.memset(adst_buf[1][:], 0.0)

        # DRAM scratch
        lin_local = dpool.tile([NPC, TW], BF, tag="lin_local")
        tables = [dpool.tile([N, TW], BF, tag=f"table{li}", name=f"table{li}",
                             addr_space="Shared") for li in range(4)]
        hres = [dpool.tile([NPC, D], BF, tag=f"hres{i}", name=f"hres{i}")
                for i in range(2)]
        hfin_local = dpool.tile([NPC, D], BF, tag="hfin_local")
        hfin_table = dpool.tile([N, D], BF, tag="hfin_table", addr_space="Shared")
        M_dram = dpool.tile([P, cfg.SK, P], F8, tag="M_dram")
        MT_dram = dpool.tile([P, cfg.SK, P], F8, tag="MT_dram")

        with tc.tile_pool(name="win", bufs=2) as win, \
             tc.tile_pool(name="psA", bufs=2, space="PSUM") as psA, \
             tc.tile_pool(name="psN", bufs=2, space="PSUM") as psN, \
             tc.tile_pool(name="psL", bufs=2, space="PSUM") as psL, \
             tc.tile_pool(name="psT", bufs=2, space="PSUM") as psT:

            def phaseA_tail(nl, w, lin_ps, cnt):
                """From lin_ps (PSUM f32 [P,256], rows :cnt valid) of layer nl,
                compute lin_sb + a_src cols, a_dst col block; store lin_local."""
                n0 = w * P
                Hn = HS[nl]
                lin_sb = win.tile([P, TW], BF, tag="lin_sb")
                nc.vector.tensor_copy(lin_sb[:cnt, 0:D], lin_ps[:cnt])
                tmp = win.tile([P, D], BF, tag="tmpA")
                nc.vector.tensor_tensor(out=tmp[:cnt], in0=lin_sb[:cnt, 0:D],
                                        in1=asrc_b[nl][:cnt], op=OP.mult)
                ad = adst_buf[nl % 2]
                with nc.allow_low_precision(reason="a_src/a_dst are tiny sums"):
                    if Hn == 1:
                        nc.vector.reduce_sum(out=lin_sb[:cnt, D:D + 1],
                                             in_=tmp[:cnt],
                                             axis=mybir.AxisListType.X)
                    else:
                        nc.vector.reduce_sum(
                            out=lin_sb[:cnt, D:D + Hn],
                            in_=tmp[:cnt].rearrange("p (c h) -> p h c", c=D // Hn),
                            axis=mybir.AxisListType.X)
                    nc.vector.tensor_tensor(out=tmp[:cnt], in0=lin_sb[:cnt, 0:D],
                                            in1=adst_b[nl][:cnt], op=OP.mult)
                    if Hn == 1:
                        nc.vector.reduce_sum(out=ad[:cnt, w * 8:w * 8 + 1],
                                             in_=tmp[:cnt],
                                             axis=mybir.AxisListType.X)
                    else:
                        nc.vector.reduce_sum(
                            out=ad[:cnt, w * 8:w * 8 + Hn],
                            in_=tmp[:cnt].rearrange("p (c h) -> p h c", c=D // Hn),
                            axis=mybir.AxisListType.X)
                nc.sync.dma_start(lin_local[n0:n0 + cnt, 0:D + Hn],
                                  lin_sb[:cnt, 0:D + Hn])

            # ---------------- prologue: phase A of layer 0 ----------------
            for w in range(W):
                n0 = w * P
                cnt = min(P, NPC - n0)
                xTw = win.tile([ND, P], BF, tag="xTw")
                nc.sync.dma_start(xTw[:, :cnt], TT["xT"][:, n0:n0 + cnt])
                lin_ps = psA.tile([P, D], F32, tag="lin_ps")
                nc.tensor.matmul(lin_ps[:cnt], lhsT=xTw[:, :cnt], rhs=W0_sb[:],
                                 start=True, stop=True)
                phaseA_tail(0, w, lin_ps, cnt)

            # ------- one-time build of fp8 indicator matrices in DRAM -------
            # (overlaps with the first AllGather)
            for w in range(W):
                K = KS[w]
                E0 = int(OFFS[w])
                dcol_sb = win.tile([P, KM], BF, tag="dcol_sb")
                nc.sync.dma_start(dcol_sb[:, :K], TT["dcol_in"][:, E0:E0 + K])
                M_b = win.tile([P, KM, P], F8, tag="M_b")
                nc.vector.tensor_tensor(
                    out=M_b[:, :K, :],
                    in0=dcol_sb[:, :K, None].to_broadcast([P, K, P]),
                    in1=iota_sb[:, None, :].to_broadcast([P, K, P]),
                    op=OP.is_equal)
                nc.sync.dma_start(M_dram[:, E0:E0 + K, :], M_b[:, :K, :])
                drow_full = win.tile([P, KM * P], BF, tag="drow_full")
                nc.sync.dma_start(
                    drow_full[:, :K * P],
                    TT["drow_in"][:, E0 * P:(E0 + K) * P].to_broadcast([P, K * P]))
                MT_b = win.tile([P, KM, P], F8, tag="MT_b")
                nc.vector.tensor_tensor(
                    out=MT_b[:, :K, :].rearrange("p k e -> p (k e)"),
                    in0=iotac_sb[:].to_broadcast([P, K * P]),
                    in1=drow_full[:, :K * P],
                    op=OP.is_equal)
                nc.sync.dma_start(MT_dram[:, E0:E0 + K, :], MT_b[:, :K, :])

            # ---------------- GAT layers ----------------
            for li in range(4):
                H = HS[li]
                DN = TW if H == 8 else D + 1
                if cfg.FAKE_AG:
                    nc.sync.dma_start(tables[li][0:NPC, :], lin_local[:])
                else:
                    nc.gpsimd.collective_compute(
                        "AllGather", OP.bypass, replica_groups=RG,
                        ins=[lin_local[:]], outs=[tables[li][:]])
                adst_cur = adst_buf[li % 2]

                for w in range(W):
                    n0 = w * P
                    cnt = min(P, NPC - n0)
                    K = KS[w]
                    E0 = int(OFFS[w])

                    idx_sb = win.tile([P, KM], I32, tag="idx_sb")
                    nc.sync.dma_start(idx_sb[:, :K], TT["idx_in"][:, E0:E0 + K])
                    M_sb = win.tile([P, KM, P], F8, tag="M_sb")
                    MT_sb = win.tile([P, KM, P], F8, tag="MT_sb")
                    if not cfg.SKIP_MMT:
                        nc.sync.dma_start(M_sb[:, :K, :], M_dram[:, E0:E0 + K, :])
                        nc.sync.dma_start(MT_sb[:, :K, :], MT_dram[:, E0:E0 + K, :])
                    aE_sb = win.tile([P, KM, 8], BF, tag="aE_sb")
                    nc.sync.dma_start(aE_sb[:, :K, :H], TT[f"aE{li}"][:, E0:E0 + K, :])

                    lin_g = win.tile([P, KM, TW], BF, tag="lin_g")
                    if cfg.SKIP_GATHER:
                        nc.vector.memset(lin_g[:], 0.0)
                    else:
                        for k in range(K):
                            nc.gpsimd.indirect_dma_start(
                                out=lin_g[:, k, :], out_offset=None,
                                in_=tables[li][:],
                                in_offset=bass.IndirectOffsetOnAxis(
                                    ap=idx_sb[:, k:k + 1], axis=0))

                    # alpha = lrelu(a_src + a_dst + a_edge, 0.2); ex = exp
                    al_ps = psL.tile([P, KM * 8], F32, tag="al_ps")
                    for k in range(K):
                        nc.tensor.matmul(al_ps[:, k * 8:k * 8 + H],
                                         lhsT=MT_sb[:, k, :],
                                         rhs=adst_cur[:, w * 8:w * 8 + H],
                                         start=True, stop=True)
                    al_sb = win.tile([P, KM, 8], BF, tag="al_sb")
                    nc.vector.tensor_tensor(
                        out=al_sb[:, :K, :H],
                        in0=al_ps[:].rearrange("p (k h) -> p k h", k=KM)[:, :K, :H],
                        in1=aE_sb[:, :K, :H], op=OP.add)
                    nc.vector.tensor_tensor(
                        out=al_sb[:, :K, :H], in0=al_sb[:, :K, :H],
                        in1=lin_g[:, :K, D:D + H], op=OP.add)
                    lr_sb = win.tile([P, KM, 8], BF, tag="lr_sb")
                    nc.vector.tensor_scalar_mul(lr_sb[:, :K, :H],
                                                al_sb[:, :K, :H], 0.2)
                    nc.vector.tensor_tensor(out=lr_sb[:, :K, :H],
                                            in0=lr_sb[:, :K, :H],
                                            in1=al_sb[:, :K, :H], op=OP.max)
                    ex_sb = win.tile([P, KM, 8], BF, tag="ex_sb")
                    nc.scalar.activation(ex_sb[:, :K, :H], lr_sb[:, :K, :H], AF.Exp)
                    if H == 1:
                        exu = win.tile([P, KM, 8], BF, tag="exu")
                        nc.vector.tensor_copy(
                            exu[:, :K, :],
                            ex_sb[:, :K, 0:1].to_broadcast([P, K, 8]))
                    else:
                        exu = ex_sb

                    # wfex = lin_g * ex (broadcast over c; h innermost -> 2x)
                    wfex = win.tile([P, KM, TW], BF, tag="wfex")
                    if not cfg.SKIP_WFEX:
                        nc.vector.tensor_tensor(
                            out=wfex[:, :K, 0:D].rearrange("p k (c h) -> p k c h", c=32),
                            in0=lin_g[:, :K, 0:D].rearrange("p k (c h) -> p k c h", c=32),
                            in1=exu[:, :K, None, :].to_broadcast([P, K, 32, 8]),
                            op=OP.mult)
                    if H == 8:
                        nc.vector.tensor_copy(wfex[:, :K, D:D + 8], exu[:, :K, :])
                    else:
                        nc.vector.tensor_copy(wfex[:, :K, D:D + 1], exu[:, :K, 0:1])

                    nu_ps = psN.tile([P, TW], F32, tag="nu_ps")
                    if cfg.SKIP_NU:
                        nc.tensor.matmul(nu_ps[:, 0:DN], lhsT=M_sb[:, 0, :],
                                         rhs=wfex[:, 0, 0:DN],
                                         start=True, stop=True)
                    else:
                        for k in range(K):
                            nc.tensor.matmul(nu_ps[:, 0:DN], lhsT=M_sb[:, k, :],
                                             rhs=wfex[:, k, 0:DN],
                                             start=(k == 0), stop=(k == K - 1))

                    # normalize, bias, ELU, residual
                    den = win.tile([P, 8], F32, tag="den")
                    nc.vector.tensor_scalar_add(den[:cnt, :H],
                                                nu_ps[:cnt, D:D + H], 1e-16)
                    rec = win.tile([P, 8], F32, tag="rec")
                    nc.vector.reciprocal(rec[:cnt, :H], den[:cnt, :H])
                    outw = win.tile([P, D], BF, tag="outw")
                    if H == 8:
                        nc.vector.tensor_tensor(
                            out=outw[:cnt].rearrange("p (c h) -> p c h", c=32),
                            in0=nu_ps[:cnt, 0:D].rearrange("p (c h) -> p c h", c=32),
                            in1=rec[:cnt, None, :H].to_broadcast([cnt, 32, 8]),
                            op=OP.mult)
                    else:
                        nc.vector.tensor_tensor(
                            out=outw[:cnt], in0=nu_ps[:cnt, 0:D],
                            in1=rec[:cnt, 0:1].to_broadcast([cnt, D]),
                            op=OP.mult)
                    nc.vector.tensor_tensor(out=outw[:cnt], in0=outw[:cnt],
                                            in1=bias_b[li][:cnt], op=OP.add)
                    # ELU = relu(x) + exp(min(x,0)) - 1
                    tmin = win.tile([P, D], BF, tag="tmin")
                    nc.vector.tensor_scalar_min(tmin[:cnt], outw[:cnt], 0.0)
                    nc.scalar.activation(tmin[:cnt], tmin[:cnt], AF.Exp)
                    trel = win.tile([P, D], BF, tag="trel")
                    nc.vector.tensor_scalar_max(trel[:cnt], outw[:cnt], 0.0)
                    hn = win.tile([P, D], BF, tag="hn")
                    if cnt < P:
                        nc.vector.memset(hn[:], 0.0)
                    nc.vector.tensor_tensor(out=hn[:cnt], in0=tmin[:cnt],
                                            in1=trel[:cnt], op=OP.add)
                    nc.vector.tensor_scalar_add(hn[:cnt], hn[:cnt], -1.0)
                    if li > 0:
                        hp = win.tile([P, D], BF, tag="hp")
                        nc.sync.dma_start(hp[:cnt], hres[(li - 1) % 2][n0:n0 + cnt])
                        if li == 3:
                            # hp is stored (c-major) space; hn is natural space
                            nc.vector.tensor_tensor(
                                out=hn[:cnt].rearrange("p (h c) -> p h c", h=8),
                                in0=hn[:cnt].rearrange("p (h c) -> p h c", h=8),
                                in1=hp[:cnt].rearrange("p (c h) -> p h c", c=32),
                                op=OP.add)
                        else:
                            nc.vector.tensor_tensor(out=hn[:cnt], in0=hn[:cnt],
                                                    in1=hp[:cnt], op=OP.add)
                    if li == 3:
                        nc.sync.dma_start(hfin_local[n0:n0 + cnt], hn[:cnt])
                        continue
                    nc.sync.dma_start(hres[li % 2][n0:n0 + cnt], hn[:cnt])

                    # fused phase A of layer li+1
                    nl = li + 1
                    trc = win.tile([P, 2 * P], BF, tag="trc")
                    for c2 in range(2):
                        tr_ps = psT.tile([P, P], BF, tag="tr_ps")
                        nc.tensor.transpose(tr_ps[:], hn[:, c2 * P:(c2 + 1) * P],
                                            ident_sb[:])
                        nc.vector.tensor_copy(trc[:, c2 * P:(c2 + 1) * P], tr_ps[:])
                    lin_ps = psA.tile([P, D], F32, tag="lin_ps")
                    for c2 in range(2):
                        nc.tensor.matmul(
                            lin_ps[:cnt],
                            lhsT=trc[:, c2 * P:c2 * P + cnt],
                            rhs=W_sb[nl][:, c2 * D:(c2 + 1) * D],
                            start=(c2 == 0), stop=(c2 == 1))
                    phaseA_tail(nl, w, lin_ps, cnt)

            # final AllGather of node features for set2set
            if cfg.FAKE_AG:
                nc.sync.dma_start(hfin_table[0:NPC, :], hfin_local[:])
            else:
                nc.gpsimd.collective_compute(
                    "AllGather", OP.bypass, replica_groups=RG,
                    ins=[hfin_local[:]], outs=[hfin_table[:]])

        build_s2s(nc, tc, cfg, TT, pers, hfin_table, ident_sb, ones_sb)


def build_s2s(nc, tc, cfg, TT, pers, hfin_table, ident_sb, ones_sb):
    NPC, GPC, TS = cfg.NPC, cfg.GPC, cfg.TS
    D, GD = cfg.D, cfg.GD
    GG = GPC
    STEPS = cfg.S2S_STEPS

    with tc.tile_pool(name="s2s", bufs=1) as sp, \
         tc.tile_pool(name="ps2", bufs=1, space="PSUM") as ps2:
        s2s_idx = sp.tile([P, TS], I32, tag="s2s_idx")
        nc.sync.dma_start(s2s_idx[:], TT["s2s_idx_in"][:])
        xn = sp.tile([P, TS, D], BF, tag="xn")
        for t in range(TS):
            nc.gpsimd.indirect_dma_start(
                out=xn[:, t, :], out_offset=None, in_=hfin_table[:],
                in_offset=bass.IndirectOffsetOnAxis(ap=s2s_idx[:, t:t + 1], axis=0))
        Mb = sp.tile([P, TS * GG], F8, tag="Mb")
        nc.sync.dma_start(Mb[:], TT["Mb_in"][:])
        MbT = sp.tile([GG, TS * P], F8, tag="MbT")
        nc.sync.dma_start(MbT[:], TT["MbT_in"][:])

        wih = sp.tile([P, 4 * 4 * D], BF, tag="wih")
        for c2 in range(4):
            nc.sync.dma_start(wih[:, c2 * 4 * D:(c2 + 1) * 4 * D],
                              TT["WihT"][c2 * P:(c2 + 1) * P, :])
        whh = sp.tile([P, 2 * 4 * D], BF, tag="whh")
        for c2 in range(2):
            nc.sync.dma_start(whh[:, c2 * 4 * D:(c2 + 1) * 4 * D],
                              TT["WhhT"][c2 * P:(c2 + 1) * P, :])
        s2sb = sp.tile([1, 4 * D], BF, tag="s2sb")
        nc.sync.dma_start(s2sb[:], TT["s2s_bias"][:])

        qT = [sp.tile([P, GG], BF, tag=f"qT{c2}", name=f"qT{c2}") for c2 in range(4)]
        c_st = sp.tile([GG, D], F32, tag="c_st")
        for t_ in qT:
            nc.vector.memset(t_[:], 0.0)
        nc.vector.memset(c_st[:], 0.0)

        gact = [AF.Sigmoid, AF.Sigmoid, AF.Tanh, AF.Sigmoid]  # i, f, g, o
        for step in range(STEPS):
            gs = []
            for g in range(4):
                g_ps = ps2.tile([GG, D], F32, tag="psY")
                nc.tensor.matmul(g_ps[:], lhsT=ones_sb[:, 0:GG],
                                 rhs=s2sb[:, g * D:(g + 1) * D],
                                 start=True, stop=False)
                for c2 in range(4):
                    nc.tensor.matmul(
                        g_ps[:], lhsT=qT[c2][:],
                        rhs=wih[:, c2 * 4 * D + g * D: c2 * 4 * D + (g + 1) * D],
                        start=False, stop=False)
                for c2 in range(2):
                    nc.tensor.matmul(
                        g_ps[:], lhsT=qT[c2][:],
                        rhs=whh[:, c2 * 4 * D + g * D: c2 * 4 * D + (g + 1) * D],
                        start=False, stop=(c2 == 1))
                g_sb = sp.tile([GG, D], F32, tag=f"g_sb{g}")
                nc.scalar.activation(g_sb[:], g_ps[:], gact[g])
                gs.append(g_sb)
            t1 = sp.tile([GG, D], F32, tag="t1")
            nc.vector.tensor_tensor(out=t1[:], in0=gs[0][:], in1=gs[2][:], op=OP.mult)
            nc.vector.tensor_tensor(out=c_st[:], in0=gs[1][:], in1=c_st[:], op=OP.mult)
            nc.vector.tensor_tensor(out=c_st[:], in0=c_st[:], in1=t1[:], op=OP.add)
            tc_sb = sp.tile([GG, D], F32, tag="tc_sb")
            nc.scalar.activation(tc_sb[:], c_st[:], AF.Tanh)
            h_l = sp.tile([GG, D], BF, tag="h_l")
            nc.vector.tensor_tensor(out=h_l[:], in0=gs[3][:], in1=tc_sb[:], op=OP.mult)

            # attention over nodes: e = <xn, h[batch]>, softmax per graph
            e_all = sp.tile([P, TS], F32, tag="e_all")
            escr = sp.tile([P, D], BF, tag="escr")
            for t in range(TS):
                he_ps = ps2.tile([P, D], F32, tag="psH")
                nc.tensor.matmul(he_ps[:], lhsT=MbT[:, t * P:(t + 1) * P],
                                 rhs=h_l[:], start=True, stop=True)
                nc.vector.tensor_tensor(out=escr[:], in0=xn[:, t, :],
                                        in1=he_ps[:], op=OP.mult)
                nc.vector.reduce_sum(out=e_all[:, t:t + 1], in_=escr[:],
                                     axis=mybir.AxisListType.X)
            eb = sp.tile([P, TS], BF, tag="eb")
            nc.scalar.activation(eb[:], e_all[:], AF.Exp)
            r_ps = ps2.tile([GG, D + 1], F32, tag="psR")
            for t in range(TS):
                wxex = sp.tile([P, D + 1], BF, tag="wxex")
                nc.vector.tensor_tensor(
                    out=wxex[:, 0:D], in0=xn[:, t, :],
                    in1=eb[:, t:t + 1].to_broadcast([P, D]), op=OP.mult)
                nc.vector.tensor_copy(wxex[:, D:D + 1], eb[:, t:t + 1])
                nc.tensor.matmul(r_ps[:], lhsT=Mb[:, t * GG:(t + 1) * GG],
                                 rhs=wxex[:], start=(t == 0), stop=(t == TS - 1))
            den = sp.tile([GG, 1], F32, tag="s2s_den")
            nc.vector.tensor_scalar_add(den[:], r_ps[:, D:D + 1], 1e-16)
            rec = sp.tile([GG, 1], F32, tag="s2s_rec")
            nc.vector.reciprocal(rec[:], den[:])
            r_sb = sp.tile([GG, D], BF, tag="r_sb")
            nc.vector.tensor_tensor(out=r_sb[:], in0=r_ps[:, 0:D],
                                    in1=rec[:].to_broadcast([GG, D]), op=OP.mult)
            for c2 in range(2):
                tr_ps = ps2.tile([P, GG], BF, tag="psX")
                nc.tensor.transpose(tr_ps[:], h_l[:, c2 * P:(c2 + 1) * P],
                                    ident_sb[:GG, :GG])
                nc.vector.tensor_copy(qT[c2][:], tr_ps[:])
                tr_ps2 = ps2.tile([P, GG], BF, tag="psX")
                nc.tensor.transpose(tr_ps2[:], r_sb[:, c2 * P:(c2 + 1) * P],
                                    ident_sb[:GG, :GG])
                nc.vector.tensor_copy(qT[2 + c2][:], tr_ps2[:])

        # ---------------- MLP head ----------------
        gfT_sb = sp.tile([GD, GG], BF, tag="gfT_sb")
        nc.sync.dma_start(gfT_sb[:], TT["gfT_in"][:])
        p1w_sb = sp.tile([P, 4 * D], BF, tag="p1w_sb")
        for c2 in range(4):
            nc.sync.dma_start(p1w_sb[:, c2 * D:(c2 + 1) * D],
                              TT["p1W"][c2 * P:(c2 + 1) * P, :])
        p1wg_sb = sp.tile([GD, D], BF, tag="p1wg_sb")
        nc.sync.dma_start(p1wg_sb[:], TT["p1W"][4 * P:4 * P + GD, :])
        p1b_sb = sp.tile([1, D], BF, tag="p1b_sb")
        nc.sync.dma_start(p1b_sb[:], TT["p1b"][:])
        z1_ps = ps2.tile([GG, D], F32, tag="psY")
        nc.tensor.matmul(z1_ps[:], lhsT=ones_sb[:, 0:GG], rhs=p1b_sb[:],
                         start=True, stop=False)
        for c2 in range(4):
            nc.tensor.matmul(z1_ps[:], lhsT=qT[c2][:],
                             rhs=p1w_sb[:, c2 * D:(c2 + 1) * D],
                             start=False, stop=False)
        nc.tensor.matmul(z1_ps[:], lhsT=gfT_sb[:], rhs=p1wg_sb[:],
                         start=False, stop=True)
        z1 = sp.tile([GG, D], BF, tag="z1")
        nc.scalar.activation(z1[:], z1_ps[:], AF.Relu)

        p2w_sb = sp.tile([P, 2 * (D // 2)], BF, tag="p2w_sb")
        for c2 in range(2):
            nc.sync.dma_start(p2w_sb[:, c2 * (D // 2):(c2 + 1) * (D // 2)],
                              TT["p2W"][c2 * P:(c2 + 1) * P, :])
        p2b_sb = sp.tile([1, D // 2], BF, tag="p2b_sb")
        nc.sync.dma_start(p2b_sb[:], TT["p2b"][:])
        z2_ps = ps2.tile([GG, D // 2], F32, tag="psY")
        nc.tensor.matmul(z2_ps[:], lhsT=ones_sb[:, 0:GG], rhs=p2b_sb[:],
                         start=True, stop=False)
        for c2 in range(2):
            z1T_ps = ps2.tile([P, GG], BF, tag="psX")
            nc.tensor.transpose(z1T_ps[:], z1[:, c2 * P:(c2 + 1) * P],
                                ident_sb[:GG, :GG])
            z1T = sp.tile([P, GG], BF, tag="z1T")
            nc.vector.tensor_copy(z1T[:], z1T_ps[:])
            nc.tensor.matmul(z2_ps[:], lhsT=z1T[:],
                             rhs=p2w_sb[:, c2 * (D // 2):(c2 + 1) * (D // 2)],
                             start=False, stop=(c2 == 1))
        z2 = sp.tile([GG, D // 2], BF, tag="z2")
        nc.scalar.activation(z2[:], z2_ps[:], AF.Relu)

        p3w_sb = sp.tile([D // 2, 5], BF, tag="p3w_sb")
        nc.sync.dma_start(p3w_sb[:], TT["p3W"][:])
        p3b_sb = sp.tile([1, 5], BF, tag="p3b_sb")
        nc.sync.dma_start(p3b_sb[:], TT["p3b"][:])
        z2T_ps = ps2.tile([P, GG], BF, tag="psX")
        nc.tensor.transpose(z2T_ps[:], z2[:], ident_sb[:GG, :GG])
        z2T = sp.tile([P, GG], BF, tag="z2T")
        nc.vector.tensor_copy(z2T[:], z2T_ps[:])
        o_ps = ps2.tile([GG, 5], F32, tag="psY")
        nc.tensor.matmul(o_ps[:], lhsT=ones_sb[:, 0:GG], rhs=p3b_sb[:],
                         start=True, stop=False)
        nc.tensor.matmul(o_ps[:], lhsT=z2T[:], rhs=p3w_sb[:],
                         start=False, stop=True)
        o_sb = sp.tile([GG, 5], F32, tag="o_sb")
        nc.vector.tensor_copy(o_sb[:], o_ps[:])
        nc.sync.dma_start(TT["out"][:], o_sb[:cfg.GPC])


def run_config(inputs, cfg):
    in_maps = host_prep(inputs, cfg)
    nc = build_kernel_with_maps(cfg)
    res = run_bass_kernel_spmd(nc, in_maps, core_ids=list(range(cfg.NC)))
    out = np.concatenate([res.results[c]["out"] for c in range(cfg.NC)], axis=0)
    return out.astype(np.float32)


def build_kernel_with_maps(cfg):
    return build_kernel(cfg)


def kernel(**inputs):
    return run_config(inputs, CFG.derive())
